# revision 36
# baseline (speedup 1.0000x reference)
"""Trainium2 Bass kernel for nn_CAM (channel attention module).

Reference (per batch b):
    f = x[b].reshape(N, C)                      # N = H*W = 4096, C = 512
    G = f^T f                                   # (C, C) channel gram
    A = softmax(G, axis=-1)
    out[b] = gamma * (f @ A) + x[b]

Algebraic folds:
  * residual: out[b] = f @ (gamma * A + I) -- residual add folded into the
    second matmul's moving operand.
  * symmetry: G == G^T, so only upper-triangular 128-blocks are computed
    (free dims 512/384/256/128); the 6 lower blocks are PE-transposed back.

Layout: n rows are interleaved 2-per-partition (row 256k + 2p + j lives on
partition p, slice j of chunk k).  This makes load descriptors 4KB and store
descriptors 2KB contiguous (vs 2KB/1KB non-interleaved), which measured
~25% faster stores under full 8-core HBM contention.  The gram is invariant
to the n-permutation; ft/MM2/store all use the same ordering consistently.

Schedule (per core, 2 batches, streaming):
  - chunk k of batch b arrives (SWDGE cast fp32->bf16, 16 chunks/batch)
  - DVE casts it to fp8; PE transposes it into ft (f^T) and accumulates the
    triangular gram into 4 parallel PSUM banks (fp8 DoubleRow, 256-row
    contraction per chunk)
  - after the last chunk: G rows copy to SBUF (bf16), lower blocks are
    mirrored by 6 PE transposes, then per-row softmax (DVE max / ACT exp
    with accum / DVE reciprocal+scale) produces B = gamma/s * E + I (bf16)
  - MM2: out rows = ft_chunk^T @ B accumulated over the 4 channel blocks in
    PSUM; batch 1's loads queue immediately behind batch 0's so DMA runs
    continuously, and batch 0's MM2 groups interleave with batch 1's
    stream work on the PE.  Batch 1's gram matmuls are front-loaded so its
    softmax chain hides under the deferred second half of batch 0's MM2.
  - PSUM->SBUF copies are split ACT:DVE ~2:1; stores are HWDGE (sync)
    with one 2KB-descriptor store per 256-row group.

Sharding: pure data-parallel over batch: 16 batches -> 8 cores x 2.
"""

import sys

if "/opt/trn_rl_repo" not in sys.path:
    sys.path.insert(0, "/opt/trn_rl_repo")

import numpy as np
import ml_dtypes

import concourse.bacc as bacc
import concourse.mybir as mybir
import concourse.tile as tile
from concourse.alu_op_type import AluOpType
from concourse.bass_utils import run_bass_kernel_spmd

F32 = mybir.dt.float32
BF16 = mybir.dt.bfloat16
FP8 = mybir.dt.float8e4
AF = mybir.ActivationFunctionType

N_CORES = 8
B_FULL, H, W, C = 16, 64, 64, 512
N = H * W                      # 4096 spatial positions per batch
B_LOC = B_FULL // N_CORES      # 2 batches per core
NM = C // 128                  # 4 channel blocks
NKC = N // 256                 # 16 interleaved 256-row chunks per batch


def build_nc(b_loc=B_LOC, n=N, c=C, num_devices=N_CORES, reps=None,
             ablate=None, staggered=True, act_share=3, ft_flat=True,
             mixed_load=False, **_legacy):
    """Build + compile the per-core Bass program.

    reps: if set, wrap the body in a hardware For_i loop (timing builds).
    act_share: of every act_share psum->sbuf copies, 1 goes to DVE, the
        rest to ACT.
    """
    nkc = n // 256   # interleaved 256-row chunks
    nm = c // 128

    nc = bacc.Bacc(
        "TRN2",
        target_bir_lowering=False,
        debug=False,
        num_devices=num_devices,
    )

    x_d = nc.dram_tensor("x", [b_loc * n, c], F32, kind="ExternalInput")
    gam_d = nc.dram_tensor("gamma", [1, 1], F32, kind="ExternalInput")
    id_d = nc.dram_tensor("ident", [c, c], BF16, kind="ExternalInput")
    y_d = nc.dram_tensor("y", [b_loc * n, c], BF16, kind="ExternalOutput")

    with tile.TileContext(nc) as tc:
        with (
            tc.tile_pool(name="fbc", bufs=8) as p_fb,     # bf16 chunk staging
            tc.tile_pool(name="fx", bufs=5) as p_fx,       # fp32 chunk staging
            tc.tile_pool(name="f8c", bufs=8) as p_f8,      # fp8 chunk staging
            tc.tile_pool(name="ft", bufs=2) as p_ft,       # f^T bf16 per batch
            tc.tile_pool(name="gsb", bufs=2 * nm) as p_g,
            tc.tile_pool(name="esb", bufs=2 * nm) as p_e,
            tc.tile_pool(name="bsb", bufs=2 * nm) as p_b,
            tc.tile_pool(name="stat", bufs=8 * nm) as p_stat,
            tc.tile_pool(name="outp", bufs=6) as p_out,
            tc.tile_pool(name="const", bufs=1) as p_const,
            tc.tile_pool(name="psg", bufs=1, space="PSUM") as p_psg,
            tc.tile_pool(name="pst", bufs=2, space="PSUM") as p_pst,
            tc.tile_pool(name="pso", bufs=2, space="PSUM") as p_pso,
        ):
            # --- constants (outside the timing loop) ---
            ident_rows = []
            for m in range(nm):
                t = p_const.tile([128, c], BF16, tag=f"ident{m}",
                                 name=f"ident{m}")
                nc.sync.dma_start(out=t[:, :],
                                  in_=id_d[m * 128:(m + 1) * 128, :])
                ident_rows.append(t)
            ident128 = ident_rows[0][:, 0:128]
            gam1 = p_const.tile([1, 1], F32, tag="gam1", name="gam1")
            nc.sync.dma_start(out=gam1[:, :], in_=gam_d[:, :])
            gamb = p_const.tile([128, 1], F32, tag="gamb", name="gamb")
            nc.gpsimd.partition_broadcast(gamb[:, :], gam1[:, :])

            copy_ctr = [0]

            def cp_engine():
                """Rotate psum->sbuf copies: 1 in act_share goes to DVE."""
                i = copy_ctr[0]
                copy_ctr[0] += 1
                if i % act_share == 0:
                    return nc.vector.tensor_copy
                return nc.scalar.copy

            def load_chunk(b, k):
                """Even chunks: SWDGE cast-DMA straight to bf16.  Odd
                chunks: HWDGE plain fp32 (separate DGE machinery -- the two
                queues together sustain ~1.4x one queue's load bandwidth),
                cast to bf16 by DVE/ACT in chunk_work."""
                base = b * n + k * 256
                src = x_d[base:base + 256, :].rearrange(
                    "(p j) c1 -> p j c1", p=128)
                if not mixed_load or k % 2 == 0:
                    fbk = p_fb.tile([128, 2, c], BF16, tag="fbc",
                                    name=f"fb{b}_{k}")
                    nc.gpsimd.dma_start(out=fbk[:, :, :], in_=src)
                    return fbk
                fxk = p_fx.tile([128, 2, c], F32, tag="fx",
                                name=f"fx{b}_{k}")
                nc.sync.dma_start(out=fxk[:, :, :], in_=src)
                return fxk

            def gram_part(b, k, fbk, psg_rows):
                """f8 cast + gram accumulation for chunk k."""
                f8k = p_f8.tile([128, 2, c], FP8, tag="f8c",
                                name=f"f8{b}_{k}")
                nc.vector.tensor_copy(f8k[:, :, :], fbk[:, :, :])
                # gram: one DoubleRow matmul per row-block (256-contraction)
                for m in range(nm if ablate != "nogram" else 0):
                    lo = m * 128
                    nc.tensor.matmul(
                        psg_rows[m][:, lo:c],
                        f8k[:, :, m * 128:(m + 1) * 128],
                        f8k[:, :, lo:c],
                        start=(k == 0),
                        stop=(k == nkc - 1),
                        perf_mode=mybir.MatmulPerfMode.DoubleRow,
                    )

            def tpose_part(b, k, fbk, ft):
                """8 transposes per chunk into one staging tile, one copy."""
                ps_t = p_pst.tile([128, 2, c], BF16, tag="pst",
                                  name=f"pst{b}_{k}")
                for j in range(2):
                    for m in range(nm):
                        nc.tensor.transpose(
                            ps_t[:, j, m * 128:(m + 1) * 128],
                            fbk[:, j, m * 128:(m + 1) * 128],
                            ident128,
                        )
                if ft_flat:
                    cp_engine()(ft[:, k, :, :], ps_t[:, :, :])
                else:
                    cp_engine()(
                        ft[:, :, k * 256: (k + 1) * 256]
                            .rearrange("p m (j f) -> p j m f", j=2),
                        ps_t[:, :, :].rearrange("p j (m f) -> p j m f", m=nm),
                    )

            def to_bf16(b, k, raw):
                """Cast an HWDGE fp32 chunk to bf16 (no-op for bf16)."""
                if raw.dtype != F32:
                    return raw
                fbk = p_fb.tile([128, 2, c], BF16, tag="fbb", bufs=6,
                                name=f"fbb{b}_{k}")
                eng = (nc.vector.tensor_copy if k % 4 == 1
                       else nc.scalar.copy)
                eng(fbk[:, :, :], raw[:, :, :])
                return fbk

            def chunk_work(b, k, fbk, ft, psg_rows):
                fbk = to_bf16(b, k, fbk)
                gram_part(b, k, fbk, psg_rows)
                tpose_part(b, k, fbk, ft)

            def gram_finish(b, psg_rows):
                """PSUM G -> SBUF (bf16), mirror lower blocks, softmax -> B.

                Progressive: row m's softmax is emitted as soon as its copy
                and mirror blocks are available, so b_rows[0] (and MM2) can
                start while later rows are still being assembled.
                """
                g_sb = []
                b_rows = []

                def softmax_row(m):
                    t_g = g_sb[m]
                    nmax = p_stat.tile([128, 1], F32, tag="nmax",
                                       name=f"nmax{b}_{m}")
                    nc.vector.reduce_max(
                        nmax[:, :], t_g[:, :], axis=mybir.AxisListType.X,
                        negate=True,
                    )
                    e_sb = p_e.tile([128, c], BF16, tag="esb",
                                    name=f"e{b}_{m}")
                    esum = p_stat.tile([128, 1], F32, tag="esum",
                                       name=f"esum{b}_{m}")
                    nc.scalar.activation(
                        e_sb[:, :], t_g[:, :], AF.Exp,
                        bias=nmax[:, :], scale=1.0, accum_out=esum[:, :],
                    )
                    rec = p_stat.tile([128, 1], F32, tag="rec",
                                      name=f"rec{b}_{m}")
                    nc.vector.reciprocal(rec[:, :], esum[:, :])
                    sc = p_stat.tile([128, 1], F32, tag="sc",
                                     name=f"sc{b}_{m}")
                    nc.vector.tensor_tensor(
                        sc[:, :], rec[:, :], gamb[:, :], op=AluOpType.mult,
                    )
                    b_sb = p_b.tile([128, c], BF16, tag="bsb",
                                    name=f"bmat{b}_{m}")
                    nc.vector.scalar_tensor_tensor(
                        b_sb[:, :], e_sb[:, :], sc[:, :],
                        ident_rows[m][:, :],
                        op0=AluOpType.mult, op1=AluOpType.add,
                    )
                    b_rows.append(b_sb)

                for m in range(nm):
                    lo = m * 128
                    t_g = p_g.tile([128, c], BF16, tag="gsb", name=f"g{b}_{m}")
                    eng = nc.vector.tensor_copy if m % 2 else nc.scalar.copy
                    eng(t_g[:, lo:c], psg_rows[m][:, lo:c])
                    g_sb.append(t_g)
                    # mirror blocks (d, m) -> (m, d) for d < m
                    for d in range(m):
                        tp = p_pst.tile([128, 2, c], BF16, tag="pst",
                                        name=f"gt{b}_{m}_{d}")
                        nc.tensor.transpose(
                            tp[:, 0, 0:128],
                            g_sb[d][:, m * 128:(m + 1) * 128],
                            ident128,
                        )
                        nc.vector.tensor_copy(
                            t_g[:, d * 128:(d + 1) * 128], tp[:, 0, 0:128])
                    softmax_row(m)
                return b_rows

            def mm2_group(b, k, ft, b_rows):
                """Second matmul + store for one 256-row group."""
                o_sb = p_out.tile([128, 2, c], BF16, tag="outp",
                                  name=f"o{b}_{k}")
                for j in range(2):
                    ps_o = p_pso.tile([128, c], F32, tag="pso",
                                      name=f"pso{b}_{k}_{j}")
                    for m in range(nm):
                        stat = (ft[:, k, j, m * 128:(m + 1) * 128] if ft_flat
                                else ft[:, m, k * 256 + j * 128:
                                        k * 256 + (j + 1) * 128])
                        nc.tensor.matmul(
                            ps_o[:, :],
                            stat,
                            b_rows[m][:, :],
                            start=(m == 0),
                            stop=(m == nm - 1),
                        )
                    cp_engine()(o_sb[:, j, :], ps_o[:, :])
                base = b * n + k * 256
                nc.sync.dma_start(
                    out=y_d[base:base + 256, :]
                        .rearrange("(p j) c1 -> p j c1", p=128),
                    in_=o_sb[:, :, :],
                )

            def body(_iv=None):
                # ---- batch 0 stream ----
                ftshape = ([128, nkc, 2, nm * 128] if ft_flat
                           else [128, nm, n])
                ft0 = p_ft.tile(ftshape, BF16, tag="ft", name="ft0")
                psg0 = [p_psg.tile([128, c], F32, tag=f"psg{m}",
                                   name=f"psg0_{m}") for m in range(nm)]
                fb0 = [load_chunk(0, k) for k in range(nkc)]
                if ablate == "loads":
                    for k in range(nkc):
                        # keep a reader so tiles are consumed
                        f8k = p_f8.tile([128, 2, c], FP8, tag="f8c",
                                        name=f"f8d0_{k}")
                        nc.vector.tensor_copy(f8k[:, :, :], fb0[k][:, :, :])
                    return
                # batch 1 loads queue right behind batch 0's: the SWDGE queue
                # runs continuously at full HBM rate while compute trails.
                fb1 = [load_chunk(1, k) for k in range(nkc)]
                for k in range(nkc):
                    chunk_work(0, k, fb0[k], ft0, psg0)
                if ablate in ("nogram", "nofinish"):
                    return
                b_rows0 = gram_finish(0, psg0)
                if ablate == "gram":
                    return

                # ---- batch 1 stream interleaved with batch 0 MM2 ----
                ft1 = p_ft.tile(ftshape, BF16, tag="ft", name="ft1")
                psg1 = [p_psg.tile([128, c], F32, tag=f"psg{m}",
                                   name=f"psg1_{m}") for m in range(nm)]
                # batch 1 gram parts are front-loaded 2-per-unit so gram 1
                # finishes ~when its last chunk lands; its softmax chain then
                # hides under the deferred second half of batch 0's MM2.
                for k in range(nkc // 2):
                    ca = to_bf16(1, 2 * k, fb1[2 * k])
                    cb = to_bf16(1, 2 * k + 1, fb1[2 * k + 1])
                    gram_part(1, 2 * k, ca, psg1)
                    gram_part(1, 2 * k + 1, cb, psg1)
                    mm2_group(0, k, ft0, b_rows0)
                    tpose_part(1, 2 * k, ca, ft1)
                    tpose_part(1, 2 * k + 1, cb, ft1)
                if ablate == "phase4":
                    return
                b_rows1 = gram_finish(1, psg1)
                for k in range(nkc // 2, nkc):
                    mm2_group(0, k, ft0, b_rows0)
                if ablate == "fin1":
                    return
                for k in range(nkc):
                    mm2_group(1, k, ft1, b_rows1)

            if reps is None:
                body()
            else:
                with tc.For_i(0, reps, 1,
                              staggered_reset=staggered,
                              hint_engines=(mybir.EngineType.PE,
                                            mybir.EngineType.DVE,
                                            mybir.EngineType.Activation,
                                            mybir.EngineType.Pool,
                                            mybir.EngineType.SP)) as iv:
                    body(iv)

    nc.compile()
    return nc


_NC_CACHE = {}


def _get_nc():
    if "full" not in _NC_CACHE:
        _NC_CACHE["full"] = build_nc()
    return _NC_CACHE["full"]


def make_in_maps(inputs_np, gamma_np):
    """Shard full inputs into per-core in_maps."""
    x = np.ascontiguousarray(
        np.asarray(inputs_np, dtype=np.float32).reshape(B_FULL, N, C)
    )
    gam = np.asarray(gamma_np, dtype=np.float32).reshape(1, 1)
    ident = np.eye(C, dtype=np.float32).astype(ml_dtypes.bfloat16)
    in_maps = []
    for core in range(N_CORES):
        xs = x[core * B_LOC:(core + 1) * B_LOC].reshape(B_LOC * N, C)
        in_maps.append({
            "x": np.ascontiguousarray(xs),
            "gamma": gam,
            "ident": ident,
        })
    return in_maps


def kernel(inputs, gamma):
    nc = _get_nc()
    in_maps = make_in_maps(inputs, gamma)
    res = run_bass_kernel_spmd(nc, in_maps, core_ids=list(range(N_CORES)))
    outs = [np.asarray(res.results[c]["y"], dtype=np.float32)
            .reshape(B_LOC, N, C) for c in range(N_CORES)]
    y = np.concatenate(outs, axis=0).reshape(B_FULL, H, W, C)
    return y.astype(np.float32)


# revision 39
# speedup vs baseline: 1.0603x; 1.0603x over previous
"""Trainium2 Bass kernel for nn_CAM (channel attention module).

Reference (per batch b):
    f = x[b].reshape(N, C)                      # N = H*W = 4096, C = 512
    G = f^T f                                   # (C, C) channel gram
    A = softmax(G, axis=-1)
    out[b] = gamma * (f @ A) + x[b]

Algebraic folds:
  * residual: out[b] = f @ (gamma * A + I) -- residual add folded into the
    second matmul's moving operand.
  * symmetry: G == G^T, so only upper-triangular 128-blocks are computed
    (free dims 512/384/256/128); the 6 lower blocks are PE-transposed back.

Layout: n rows are interleaved 2-per-partition (row 256k + 2p + j lives on
partition p, slice j of chunk k).  This makes load descriptors 4KB and store
descriptors 2KB contiguous (vs 2KB/1KB non-interleaved), which measured
~25% faster stores under full 8-core HBM contention.  The gram is invariant
to the n-permutation; ft/MM2/store all use the same ordering consistently.

Schedule (per core, 2 batches, streaming):
  - chunk k of batch b arrives (SWDGE cast fp32->bf16, 16 chunks/batch)
  - DVE casts it to fp8; PE transposes it into ft (f^T) and accumulates the
    triangular gram into 4 parallel PSUM banks (fp8 DoubleRow, 256-row
    contraction per chunk)
  - after the last chunk: G rows copy to SBUF (bf16), lower blocks are
    mirrored by 6 PE transposes, then per-row softmax (DVE max / ACT exp
    with accum / DVE reciprocal+scale) produces B = gamma/s * E + I (bf16)
  - MM2: out rows = ft_chunk^T @ B accumulated over the 4 channel blocks in
    PSUM; batch 1's loads queue immediately behind batch 0's so DMA runs
    continuously, and batch 0's MM2 groups interleave with batch 1's
    stream work on the PE.  Batch 1's gram matmuls are front-loaded so its
    softmax chain hides under the deferred second half of batch 0's MM2.
  - PSUM->SBUF copies are split ACT:DVE ~2:1; stores are HWDGE (sync)
    with one 2KB-descriptor store per 256-row group.

Sharding: pure data-parallel over batch: 16 batches -> 8 cores x 2.
"""

import sys

if "/opt/trn_rl_repo" not in sys.path:
    sys.path.insert(0, "/opt/trn_rl_repo")

import numpy as np
import ml_dtypes

import concourse.bacc as bacc
import concourse.mybir as mybir
import concourse.tile as tile
from concourse.alu_op_type import AluOpType
from concourse.bass_utils import run_bass_kernel_spmd

F32 = mybir.dt.float32
BF16 = mybir.dt.bfloat16
FP8 = mybir.dt.float8e4
AF = mybir.ActivationFunctionType

N_CORES = 8
B_FULL, H, W, C = 16, 64, 64, 512
N = H * W                      # 4096 spatial positions per batch
B_LOC = B_FULL // N_CORES      # 2 batches per core
NM = C // 128                  # 4 channel blocks
NKC = N // 256                 # 16 interleaved 256-row chunks per batch


def build_nc(b_loc=B_LOC, n=N, c=C, num_devices=N_CORES, reps=None,
             ablate=None, staggered=True, act_share=3, ft_flat=True,
             mixed_load=False, unroll=4, fbc_bufs=28, **_legacy):
    """Build + compile the per-core Bass program.

    reps: if set, wrap the body in a hardware For_i loop (timing builds).
    act_share: of every act_share psum->sbuf copies, 1 goes to DVE, the
        rest to ACT.
    """
    nkc = n // 256   # interleaved 256-row chunks
    nm = c // 128

    nc = bacc.Bacc(
        "TRN2",
        target_bir_lowering=False,
        debug=False,
        num_devices=num_devices,
    )

    x_d = nc.dram_tensor("x", [b_loc * n, c], F32, kind="ExternalInput")
    gam_d = nc.dram_tensor("gamma", [1, 1], F32, kind="ExternalInput")
    id_d = nc.dram_tensor("ident", [c, c], BF16, kind="ExternalInput")
    y_d = nc.dram_tensor("y", [b_loc * n, c], BF16, kind="ExternalOutput")

    with tile.TileContext(nc) as tc:
        with (
            tc.tile_pool(name="fbc", bufs=fbc_bufs) as p_fb,  # bf16 chunk staging
            tc.tile_pool(name="fx", bufs=5) as p_fx,       # fp32 chunk staging
            tc.tile_pool(name="f8c", bufs=8) as p_f8,      # fp8 chunk staging
            tc.tile_pool(name="ft", bufs=2) as p_ft,       # f^T bf16 per batch
            tc.tile_pool(name="gsb", bufs=2 * nm) as p_g,
            tc.tile_pool(name="esb", bufs=2 * nm) as p_e,
            tc.tile_pool(name="bsb", bufs=2 * nm) as p_b,
            tc.tile_pool(name="stat", bufs=8 * nm) as p_stat,
            tc.tile_pool(name="outp", bufs=6) as p_out,
            tc.tile_pool(name="const", bufs=1) as p_const,
            tc.tile_pool(name="psg", bufs=1, space="PSUM") as p_psg,
            tc.tile_pool(name="pst", bufs=2, space="PSUM") as p_pst,
            tc.tile_pool(name="pso", bufs=2, space="PSUM") as p_pso,
        ):
            # --- constants (outside the timing loop) ---
            ident_rows = []
            for m in range(nm):
                t = p_const.tile([128, c], BF16, tag=f"ident{m}",
                                 name=f"ident{m}")
                nc.sync.dma_start(out=t[:, :],
                                  in_=id_d[m * 128:(m + 1) * 128, :])
                ident_rows.append(t)
            ident128 = ident_rows[0][:, 0:128]
            gam1 = p_const.tile([1, 1], F32, tag="gam1", name="gam1")
            nc.sync.dma_start(out=gam1[:, :], in_=gam_d[:, :])
            gamb = p_const.tile([128, 1], F32, tag="gamb", name="gamb")
            nc.gpsimd.partition_broadcast(gamb[:, :], gam1[:, :])

            copy_ctr = [0]

            def cp_engine():
                """Rotate psum->sbuf copies: 1 in act_share goes to DVE."""
                i = copy_ctr[0]
                copy_ctr[0] += 1
                if i % act_share == 0:
                    return nc.vector.tensor_copy
                return nc.scalar.copy

            def load_chunk(b, k):
                """Even chunks: SWDGE cast-DMA straight to bf16.  Odd
                chunks: HWDGE plain fp32 (separate DGE machinery -- the two
                queues together sustain ~1.4x one queue's load bandwidth),
                cast to bf16 by DVE/ACT in chunk_work."""
                base = b * n + k * 256
                src = x_d[base:base + 256, :].rearrange(
                    "(p j) c1 -> p j c1", p=128)
                if not mixed_load or k % 2 == 0:
                    fbk = p_fb.tile([128, 2, c], BF16, tag="fbc",
                                    name=f"fb{b}_{k}")
                    nc.gpsimd.dma_start(out=fbk[:, :, :], in_=src)
                    return fbk
                fxk = p_fx.tile([128, 2, c], F32, tag="fx",
                                name=f"fx{b}_{k}")
                nc.sync.dma_start(out=fxk[:, :, :], in_=src)
                return fxk

            def gram_part(b, k, fbk, psg_rows):
                """f8 cast + gram accumulation for chunk k."""
                f8k = p_f8.tile([128, 2, c], FP8, tag="f8c",
                                name=f"f8{b}_{k}")
                nc.vector.tensor_copy(f8k[:, :, :], fbk[:, :, :])
                # gram: one DoubleRow matmul per row-block (256-contraction)
                for m in range(nm if ablate != "nogram" else 0):
                    lo = m * 128
                    nc.tensor.matmul(
                        psg_rows[m][:, lo:c],
                        f8k[:, :, m * 128:(m + 1) * 128],
                        f8k[:, :, lo:c],
                        start=(k == 0),
                        stop=(k == nkc - 1),
                        perf_mode=mybir.MatmulPerfMode.DoubleRow,
                    )

            def tpose_part(b, k, fbk, ft):
                """8 transposes per chunk into one staging tile, one copy."""
                ps_t = p_pst.tile([128, 2, c], BF16, tag="pst",
                                  name=f"pst{b}_{k}")
                for j in range(2):
                    for m in range(nm):
                        nc.tensor.transpose(
                            ps_t[:, j, m * 128:(m + 1) * 128],
                            fbk[:, j, m * 128:(m + 1) * 128],
                            ident128,
                        )
                if ft_flat:
                    cp_engine()(ft[:, k, :, :], ps_t[:, :, :])
                else:
                    cp_engine()(
                        ft[:, :, k * 256: (k + 1) * 256]
                            .rearrange("p m (j f) -> p j m f", j=2),
                        ps_t[:, :, :].rearrange("p j (m f) -> p j m f", m=nm),
                    )

            def to_bf16(b, k, raw):
                """Cast an HWDGE fp32 chunk to bf16 (no-op for bf16)."""
                if raw.dtype != F32:
                    return raw
                fbk = p_fb.tile([128, 2, c], BF16, tag="fbb", bufs=6,
                                name=f"fbb{b}_{k}")
                eng = (nc.vector.tensor_copy if k % 4 == 1
                       else nc.scalar.copy)
                eng(fbk[:, :, :], raw[:, :, :])
                return fbk

            def chunk_work(b, k, fbk, ft, psg_rows):
                fbk = to_bf16(b, k, fbk)
                gram_part(b, k, fbk, psg_rows)
                tpose_part(b, k, fbk, ft)

            def gram_finish(b, psg_rows):
                """PSUM G -> SBUF (bf16), mirror lower blocks, softmax -> B.

                Progressive: row m's softmax is emitted as soon as its copy
                and mirror blocks are available, so b_rows[0] (and MM2) can
                start while later rows are still being assembled.
                """
                g_sb = []
                b_rows = []

                def softmax_row(m):
                    t_g = g_sb[m]
                    nmax = p_stat.tile([128, 1], F32, tag="nmax",
                                       name=f"nmax{b}_{m}")
                    nc.vector.reduce_max(
                        nmax[:, :], t_g[:, :], axis=mybir.AxisListType.X,
                        negate=True,
                    )
                    e_sb = p_e.tile([128, c], BF16, tag="esb",
                                    name=f"e{b}_{m}")
                    esum = p_stat.tile([128, 1], F32, tag="esum",
                                       name=f"esum{b}_{m}")
                    nc.scalar.activation(
                        e_sb[:, :], t_g[:, :], AF.Exp,
                        bias=nmax[:, :], scale=1.0, accum_out=esum[:, :],
                    )
                    rec = p_stat.tile([128, 1], F32, tag="rec",
                                      name=f"rec{b}_{m}")
                    nc.vector.reciprocal(rec[:, :], esum[:, :])
                    sc = p_stat.tile([128, 1], F32, tag="sc",
                                     name=f"sc{b}_{m}")
                    nc.vector.tensor_tensor(
                        sc[:, :], rec[:, :], gamb[:, :], op=AluOpType.mult,
                    )
                    b_sb = p_b.tile([128, c], BF16, tag="bsb",
                                    name=f"bmat{b}_{m}")
                    nc.vector.scalar_tensor_tensor(
                        b_sb[:, :], e_sb[:, :], sc[:, :],
                        ident_rows[m][:, :],
                        op0=AluOpType.mult, op1=AluOpType.add,
                    )
                    b_rows.append(b_sb)

                for m in range(nm):
                    lo = m * 128
                    t_g = p_g.tile([128, c], BF16, tag="gsb", name=f"g{b}_{m}")
                    eng = nc.vector.tensor_copy if m % 2 else nc.scalar.copy
                    eng(t_g[:, lo:c], psg_rows[m][:, lo:c])
                    g_sb.append(t_g)
                    # mirror blocks (d, m) -> (m, d) for d < m
                    for d in range(m):
                        tp = p_pst.tile([128, 2, c], BF16, tag="pst",
                                        name=f"gt{b}_{m}_{d}")
                        nc.tensor.transpose(
                            tp[:, 0, 0:128],
                            g_sb[d][:, m * 128:(m + 1) * 128],
                            ident128,
                        )
                        nc.vector.tensor_copy(
                            t_g[:, d * 128:(d + 1) * 128], tp[:, 0, 0:128])
                    softmax_row(m)
                return b_rows

            def mm2_group(b, k, ft, b_rows):
                """Second matmul + store for one 256-row group."""
                o_sb = p_out.tile([128, 2, c], BF16, tag="outp",
                                  name=f"o{b}_{k}")
                for j in range(2):
                    ps_o = p_pso.tile([128, c], F32, tag="pso",
                                      name=f"pso{b}_{k}_{j}")
                    for m in range(nm):
                        stat = (ft[:, k, j, m * 128:(m + 1) * 128] if ft_flat
                                else ft[:, m, k * 256 + j * 128:
                                        k * 256 + (j + 1) * 128])
                        nc.tensor.matmul(
                            ps_o[:, :],
                            stat,
                            b_rows[m][:, :],
                            start=(m == 0),
                            stop=(m == nm - 1),
                        )
                    cp_engine()(o_sb[:, j, :], ps_o[:, :])
                base = b * n + k * 256
                nc.sync.dma_start(
                    out=y_d[base:base + 256, :]
                        .rearrange("(p j) c1 -> p j c1", p=128),
                    in_=o_sb[:, :, :],
                )

            def body(_iv=None):
                # ---- batch 0 stream ----
                ftshape = ([128, nkc, 2, nm * 128] if ft_flat
                           else [128, nm, n])
                ft0 = p_ft.tile(ftshape, BF16, tag="ft", name="ft0")
                psg0 = [p_psg.tile([128, c], F32, tag=f"psg{m}",
                                   name=f"psg0_{m}") for m in range(nm)]
                fb0 = [load_chunk(0, k) for k in range(nkc)]
                if ablate == "loads":
                    for k in range(nkc):
                        # keep a reader so tiles are consumed
                        f8k = p_f8.tile([128, 2, c], FP8, tag="f8c",
                                        name=f"f8d0_{k}")
                        nc.vector.tensor_copy(f8k[:, :, :], fb0[k][:, :, :])
                    return
                # batch 1 loads queue right behind batch 0's: the SWDGE queue
                # runs continuously at full HBM rate while compute trails.
                fb1 = [load_chunk(1, k) for k in range(nkc)]
                for k in range(nkc):
                    chunk_work(0, k, fb0[k], ft0, psg0)
                if ablate in ("nogram", "nofinish"):
                    return
                b_rows0 = gram_finish(0, psg0)
                if ablate == "gram":
                    return

                # ---- batch 1 stream interleaved with batch 0 MM2 ----
                ft1 = p_ft.tile(ftshape, BF16, tag="ft", name="ft1")
                psg1 = [p_psg.tile([128, c], F32, tag=f"psg{m}",
                                   name=f"psg1_{m}") for m in range(nm)]
                # batch 1 gram parts are front-loaded 2-per-unit so gram 1
                # finishes ~when its last chunk lands; its softmax chain then
                # hides under the deferred second half of batch 0's MM2.
                for k in range(nkc // 2):
                    ca = to_bf16(1, 2 * k, fb1[2 * k])
                    cb = to_bf16(1, 2 * k + 1, fb1[2 * k + 1])
                    gram_part(1, 2 * k, ca, psg1)
                    gram_part(1, 2 * k + 1, cb, psg1)
                    mm2_group(0, k, ft0, b_rows0)
                    tpose_part(1, 2 * k, ca, ft1)
                    tpose_part(1, 2 * k + 1, cb, ft1)
                if ablate == "phase4":
                    return
                b_rows1 = gram_finish(1, psg1)
                for k in range(nkc // 2, nkc):
                    mm2_group(0, k, ft0, b_rows0)
                if ablate == "fin1":
                    return
                for k in range(nkc):
                    mm2_group(1, k, ft1, b_rows1)

            if reps is None:
                body()
            else:
                # unroll: several kernel executions per For_i trip, so the
                # all-engine reset barrier amortizes and execution i+1's
                # loads overlap execution i's drain (MM2 of batch 1).
                assert reps % unroll == 0
                with tc.For_i(0, reps // unroll, 1,
                              staggered_reset=staggered,
                              hint_engines=(mybir.EngineType.PE,
                                            mybir.EngineType.DVE,
                                            mybir.EngineType.Activation,
                                            mybir.EngineType.Pool,
                                            mybir.EngineType.SP)) as iv:
                    for _ in range(unroll):
                        body(iv)

    nc.compile()
    return nc


_NC_CACHE = {}


def _get_nc():
    if "full" not in _NC_CACHE:
        _NC_CACHE["full"] = build_nc()
    return _NC_CACHE["full"]


def make_in_maps(inputs_np, gamma_np):
    """Shard full inputs into per-core in_maps."""
    x = np.ascontiguousarray(
        np.asarray(inputs_np, dtype=np.float32).reshape(B_FULL, N, C)
    )
    gam = np.asarray(gamma_np, dtype=np.float32).reshape(1, 1)
    ident = np.eye(C, dtype=np.float32).astype(ml_dtypes.bfloat16)
    in_maps = []
    for core in range(N_CORES):
        xs = x[core * B_LOC:(core + 1) * B_LOC].reshape(B_LOC * N, C)
        in_maps.append({
            "x": np.ascontiguousarray(xs),
            "gamma": gam,
            "ident": ident,
        })
    return in_maps


def kernel(inputs, gamma):
    nc = _get_nc()
    in_maps = make_in_maps(inputs, gamma)
    res = run_bass_kernel_spmd(nc, in_maps, core_ids=list(range(N_CORES)))
    outs = [np.asarray(res.results[c]["y"], dtype=np.float32)
            .reshape(B_LOC, N, C) for c in range(N_CORES)]
    y = np.concatenate(outs, axis=0).reshape(B_FULL, H, W, C)
    return y.astype(np.float32)


# revision 41
# speedup vs baseline: 1.1683x; 1.1019x over previous
"""Trainium2 Bass kernel for nn_CAM (channel attention module).

Reference (per batch b):
    f = x[b].reshape(N, C)                      # N = H*W = 4096, C = 512
    G = f^T f                                   # (C, C) channel gram
    A = softmax(G, axis=-1)
    out[b] = gamma * (f @ A) + x[b]

Algebraic folds:
  * residual: out[b] = f @ (gamma * A + I) -- residual add folded into the
    second matmul's moving operand.
  * symmetry: G == G^T, so only upper-triangular 128-blocks are computed
    (free dims 512/384/256/128); the 6 lower blocks are PE-transposed back.

Layout: n rows are interleaved 2-per-partition (row 256k + 2p + j lives on
partition p, slice j of chunk k).  This makes load descriptors 4KB and store
descriptors 2KB contiguous (vs 2KB/1KB non-interleaved), which measured
~25% faster stores under full 8-core HBM contention.  The gram is invariant
to the n-permutation; ft/MM2/store all use the same ordering consistently.

Schedule (per core, 2 batches, streaming):
  - chunk k of batch b arrives (SWDGE cast fp32->bf16, 16 chunks/batch)
  - DVE casts it to fp8; PE transposes it into ft (f^T) and accumulates the
    triangular gram into 4 parallel PSUM banks (fp8 DoubleRow, 256-row
    contraction per chunk)
  - after the last chunk: G rows copy to SBUF (bf16), lower blocks are
    mirrored by 6 PE transposes, then per-row softmax (DVE max / ACT exp
    with accum / DVE reciprocal+scale) produces B = gamma/s * E + I (bf16)
  - MM2: out rows = ft_chunk^T @ B accumulated over the 4 channel blocks in
    PSUM; batch 1's loads queue immediately behind batch 0's so DMA runs
    continuously, and batch 0's MM2 groups interleave with batch 1's
    stream work on the PE.  Batch 1's gram matmuls are front-loaded so its
    softmax chain hides under the deferred second half of batch 0's MM2.
  - PSUM->SBUF copies are split ACT:DVE ~2:1; stores are HWDGE (sync)
    with one 2KB-descriptor store per 256-row group.

Sharding: pure data-parallel over batch: 16 batches -> 8 cores x 2.
"""

import sys

if "/opt/trn_rl_repo" not in sys.path:
    sys.path.insert(0, "/opt/trn_rl_repo")

import numpy as np
import ml_dtypes

import concourse.bacc as bacc
import concourse.mybir as mybir
import concourse.tile as tile
from concourse.alu_op_type import AluOpType
from concourse.bass_utils import run_bass_kernel_spmd

F32 = mybir.dt.float32
BF16 = mybir.dt.bfloat16
FP8 = mybir.dt.float8e4
AF = mybir.ActivationFunctionType

N_CORES = 8
B_FULL, H, W, C = 16, 64, 64, 512
N = H * W                      # 4096 spatial positions per batch
B_LOC = B_FULL // N_CORES      # 2 batches per core
NM = C // 128                  # 4 channel blocks
NKC = N // 256                 # 16 interleaved 256-row chunks per batch


def build_nc(b_loc=B_LOC, n=N, c=C, num_devices=N_CORES, reps=None,
             ablate=None, staggered=True, act_share=3, ft_flat=True,
             mixed_load=False, unroll=4, fbc_bufs=28, dve_drain=False,
             **_legacy):
    """Build + compile the per-core Bass program.

    reps: if set, wrap the body in a hardware For_i loop (timing builds).
    act_share: of every act_share psum->sbuf copies, 1 goes to DVE, the
        rest to ACT.
    """
    nkc = n // 256   # interleaved 256-row chunks
    nm = c // 128

    nc = bacc.Bacc(
        "TRN2",
        target_bir_lowering=False,
        debug=False,
        num_devices=num_devices,
    )

    x_d = nc.dram_tensor("x", [b_loc * n, c], F32, kind="ExternalInput")
    gam_d = nc.dram_tensor("gamma", [1, 1], F32, kind="ExternalInput")
    id_d = nc.dram_tensor("ident", [c, c], BF16, kind="ExternalInput")
    y_d = nc.dram_tensor("y", [b_loc * n, c], BF16, kind="ExternalOutput")

    with tile.TileContext(nc) as tc:
        with (
            tc.tile_pool(name="fbc", bufs=fbc_bufs) as p_fb,  # bf16 chunk staging
            tc.tile_pool(name="fx", bufs=5) as p_fx,       # fp32 chunk staging
            tc.tile_pool(name="f8c", bufs=12) as p_f8,      # fp8 chunk staging
            tc.tile_pool(name="ft", bufs=2) as p_ft,       # f^T bf16 per batch
            tc.tile_pool(name="gsb", bufs=2 * nm) as p_g,
            tc.tile_pool(name="esb", bufs=2 * nm) as p_e,
            tc.tile_pool(name="bsb", bufs=2 * nm) as p_b,
            tc.tile_pool(name="stat", bufs=8 * nm) as p_stat,
            tc.tile_pool(name="outp", bufs=6) as p_out,
            tc.tile_pool(name="const", bufs=1) as p_const,
            tc.tile_pool(name="psg", bufs=1, space="PSUM") as p_psg,
            tc.tile_pool(name="pst", bufs=2, space="PSUM") as p_pst,
            tc.tile_pool(name="pso", bufs=2, space="PSUM") as p_pso,
        ):
            # --- constants (outside the timing loop) ---
            ident_rows = []
            for m in range(nm):
                t = p_const.tile([128, c], BF16, tag=f"ident{m}",
                                 name=f"ident{m}")
                nc.sync.dma_start(out=t[:, :],
                                  in_=id_d[m * 128:(m + 1) * 128, :])
                ident_rows.append(t)
            ident128 = ident_rows[0][:, 0:128]
            gam1 = p_const.tile([1, 1], F32, tag="gam1", name="gam1")
            nc.sync.dma_start(out=gam1[:, :], in_=gam_d[:, :])
            gamb = p_const.tile([128, 1], F32, tag="gamb", name="gamb")
            nc.gpsimd.partition_broadcast(gamb[:, :], gam1[:, :])

            copy_ctr = [0]

            def cp_engine():
                """Rotate psum->sbuf copies: 1 in act_share goes to DVE."""
                i = copy_ctr[0]
                copy_ctr[0] += 1
                if i % act_share == 0:
                    return nc.vector.tensor_copy
                return nc.scalar.copy

            def load_chunk(b, k):
                """Even chunks: SWDGE cast-DMA straight to bf16.  Odd
                chunks: HWDGE plain fp32 (separate DGE machinery -- the two
                queues together sustain ~1.4x one queue's load bandwidth),
                cast to bf16 by DVE/ACT in chunk_work."""
                base = b * n + k * 256
                src = x_d[base:base + 256, :].rearrange(
                    "(p j) c1 -> p j c1", p=128)
                if not mixed_load or k % 2 == 0:
                    fbk = p_fb.tile([128, 2, c], BF16, tag="fbc",
                                    name=f"fb{b}_{k}")
                    nc.gpsimd.dma_start(out=fbk[:, :, :], in_=src)
                    return fbk
                fxk = p_fx.tile([128, 2, c], F32, tag="fx",
                                name=f"fx{b}_{k}")
                nc.sync.dma_start(out=fxk[:, :, :], in_=src)
                return fxk

            def gram_part(b, k, fbk, psg_rows):
                """f8 cast + gram accumulation for chunk k."""
                f8k = p_f8.tile([128, 2, c], FP8, tag="f8c",
                                name=f"f8{b}_{k}")
                nc.vector.tensor_copy(f8k[:, :, :], fbk[:, :, :])
                # gram: one DoubleRow matmul per row-block (256-contraction)
                for m in range(nm if ablate != "nogram" else 0):
                    lo = m * 128
                    nc.tensor.matmul(
                        psg_rows[m][:, lo:c],
                        f8k[:, :, m * 128:(m + 1) * 128],
                        f8k[:, :, lo:c],
                        start=(k == 0),
                        stop=(k == nkc - 1),
                        perf_mode=mybir.MatmulPerfMode.DoubleRow,
                    )

            def tpose_part(b, k, fbk, ft):
                """8 transposes per chunk into one staging tile, one copy."""
                ps_t = p_pst.tile([128, 2, c], BF16, tag="pst",
                                  name=f"pst{b}_{k}")
                for j in range(2):
                    for m in range(nm):
                        nc.tensor.transpose(
                            ps_t[:, j, m * 128:(m + 1) * 128],
                            fbk[:, j, m * 128:(m + 1) * 128],
                            ident128,
                        )
                if ft_flat:
                    cp_engine()(ft[:, k, :, :], ps_t[:, :, :])
                else:
                    cp_engine()(
                        ft[:, :, k * 256: (k + 1) * 256]
                            .rearrange("p m (j f) -> p j m f", j=2),
                        ps_t[:, :, :].rearrange("p j (m f) -> p j m f", m=nm),
                    )

            def to_bf16(b, k, raw):
                """Cast an HWDGE fp32 chunk to bf16 (no-op for bf16)."""
                if raw.dtype != F32:
                    return raw
                fbk = p_fb.tile([128, 2, c], BF16, tag="fbb", bufs=6,
                                name=f"fbb{b}_{k}")
                eng = (nc.vector.tensor_copy if k % 4 == 1
                       else nc.scalar.copy)
                eng(fbk[:, :, :], raw[:, :, :])
                return fbk

            def chunk_work(b, k, fbk, ft, psg_rows):
                fbk = to_bf16(b, k, fbk)
                gram_part(b, k, fbk, psg_rows)
                tpose_part(b, k, fbk, ft)

            def gram_finish(b, psg_rows):
                """PSUM G -> SBUF (bf16), mirror lower blocks, softmax -> B.

                Progressive: row m's softmax is emitted as soon as its copy
                and mirror blocks are available, so b_rows[0] (and MM2) can
                start while later rows are still being assembled.
                """
                g_sb = []
                b_rows = []

                def softmax_row(m):
                    t_g = g_sb[m]
                    nmax = p_stat.tile([128, 1], F32, tag="nmax",
                                       name=f"nmax{b}_{m}")
                    nc.vector.reduce_max(
                        nmax[:, :], t_g[:, :], axis=mybir.AxisListType.X,
                        negate=True,
                    )
                    e_sb = p_e.tile([128, c], BF16, tag="esb",
                                    name=f"e{b}_{m}")
                    esum = p_stat.tile([128, 1], F32, tag="esum",
                                       name=f"esum{b}_{m}")
                    nc.scalar.activation(
                        e_sb[:, :], t_g[:, :], AF.Exp,
                        bias=nmax[:, :], scale=1.0, accum_out=esum[:, :],
                    )
                    rec = p_stat.tile([128, 1], F32, tag="rec",
                                      name=f"rec{b}_{m}")
                    nc.vector.reciprocal(rec[:, :], esum[:, :])
                    sc = p_stat.tile([128, 1], F32, tag="sc",
                                     name=f"sc{b}_{m}")
                    nc.vector.tensor_tensor(
                        sc[:, :], rec[:, :], gamb[:, :], op=AluOpType.mult,
                    )
                    b_sb = p_b.tile([128, c], BF16, tag="bsb",
                                    name=f"bmat{b}_{m}")
                    nc.vector.scalar_tensor_tensor(
                        b_sb[:, :], e_sb[:, :], sc[:, :],
                        ident_rows[m][:, :],
                        op0=AluOpType.mult, op1=AluOpType.add,
                    )
                    b_rows.append(b_sb)

                for m in range(nm):
                    lo = m * 128
                    t_g = p_g.tile([128, c], BF16, tag="gsb", name=f"g{b}_{m}")
                    eng = nc.vector.tensor_copy if m % 2 else nc.scalar.copy
                    eng(t_g[:, lo:c], psg_rows[m][:, lo:c])
                    g_sb.append(t_g)
                    # mirror blocks (d, m) -> (m, d) for d < m
                    for d in range(m):
                        tp = p_pst.tile([128, 2, c], BF16, tag="pst",
                                        name=f"gt{b}_{m}_{d}")
                        nc.tensor.transpose(
                            tp[:, 0, 0:128],
                            g_sb[d][:, m * 128:(m + 1) * 128],
                            ident128,
                        )
                        nc.vector.tensor_copy(
                            t_g[:, d * 128:(d + 1) * 128], tp[:, 0, 0:128])
                    softmax_row(m)
                return b_rows

            def mm2_group(b, k, ft, b_rows, act_only=False):
                """Second matmul + store for one 256-row group.

                act_only: route copies to ACT so DVE drains early and can
                pre-cast the next execution's fp8 chunks.
                """
                o_sb = p_out.tile([128, 2, c], BF16, tag="outp",
                                  name=f"o{b}_{k}")
                for j in range(2):
                    ps_o = p_pso.tile([128, c], F32, tag="pso",
                                      name=f"pso{b}_{k}_{j}")
                    for m in range(nm):
                        stat = (ft[:, k, j, m * 128:(m + 1) * 128] if ft_flat
                                else ft[:, m, k * 256 + j * 128:
                                        k * 256 + (j + 1) * 128])
                        nc.tensor.matmul(
                            ps_o[:, :],
                            stat,
                            b_rows[m][:, :],
                            start=(m == 0),
                            stop=(m == nm - 1),
                        )
                    (nc.scalar.copy if act_only
                     else cp_engine())(o_sb[:, j, :], ps_o[:, :])
                base = b * n + k * 256
                nc.sync.dma_start(
                    out=y_d[base:base + 256, :]
                        .rearrange("(p j) c1 -> p j c1", p=128),
                    in_=o_sb[:, :, :],
                )

            def body(_iv=None):
                # ---- batch 0 stream ----
                ftshape = ([128, nkc, 2, nm * 128] if ft_flat
                           else [128, nm, n])
                ft0 = p_ft.tile(ftshape, BF16, tag="ft", name="ft0")
                psg0 = [p_psg.tile([128, c], F32, tag=f"psg{m}",
                                   name=f"psg0_{m}") for m in range(nm)]
                fb0 = [load_chunk(0, k) for k in range(nkc)]
                if ablate == "loads":
                    for k in range(nkc):
                        # keep a reader so tiles are consumed
                        f8k = p_f8.tile([128, 2, c], FP8, tag="f8c",
                                        name=f"f8d0_{k}")
                        nc.vector.tensor_copy(f8k[:, :, :], fb0[k][:, :, :])
                    return
                # batch 1 loads queue right behind batch 0's: the SWDGE queue
                # runs continuously at full HBM rate while compute trails.
                fb1 = [load_chunk(1, k) for k in range(nkc)]
                for k in range(nkc):
                    chunk_work(0, k, fb0[k], ft0, psg0)
                if ablate in ("nogram", "nofinish"):
                    return
                b_rows0 = gram_finish(0, psg0)
                if ablate == "gram":
                    return

                # ---- batch 1 stream interleaved with batch 0 MM2 ----
                ft1 = p_ft.tile(ftshape, BF16, tag="ft", name="ft1")
                psg1 = [p_psg.tile([128, c], F32, tag=f"psg{m}",
                                   name=f"psg1_{m}") for m in range(nm)]
                # batch 1 gram parts are front-loaded 2-per-unit so gram 1
                # finishes ~when its last chunk lands; its softmax chain then
                # hides under the deferred second half of batch 0's MM2.
                for k in range(nkc // 2):
                    ca = to_bf16(1, 2 * k, fb1[2 * k])
                    cb = to_bf16(1, 2 * k + 1, fb1[2 * k + 1])
                    gram_part(1, 2 * k, ca, psg1)
                    gram_part(1, 2 * k + 1, cb, psg1)
                    mm2_group(0, k, ft0, b_rows0)
                    tpose_part(1, 2 * k, ca, ft1)
                    tpose_part(1, 2 * k + 1, cb, ft1)
                if ablate == "phase4":
                    return
                b_rows1 = gram_finish(1, psg1)
                for k in range(nkc // 2, nkc):
                    mm2_group(0, k, ft0, b_rows0)
                if ablate == "fin1":
                    return
                for k in range(nkc):
                    mm2_group(1, k, ft1, b_rows1, act_only=dve_drain)

            if reps is None:
                body()
            else:
                # unroll: several kernel executions per For_i trip, so the
                # all-engine reset barrier amortizes and execution i+1's
                # loads overlap execution i's drain (MM2 of batch 1).
                assert reps % unroll == 0
                with tc.For_i(0, reps // unroll, 1,
                              staggered_reset=staggered,
                              hint_engines=(mybir.EngineType.PE,
                                            mybir.EngineType.DVE,
                                            mybir.EngineType.Activation,
                                            mybir.EngineType.Pool,
                                            mybir.EngineType.SP)) as iv:
                    for _ in range(unroll):
                        body(iv)

    nc.compile()
    return nc


_NC_CACHE = {}


def _get_nc():
    if "full" not in _NC_CACHE:
        _NC_CACHE["full"] = build_nc()
    return _NC_CACHE["full"]


def make_in_maps(inputs_np, gamma_np):
    """Shard full inputs into per-core in_maps."""
    x = np.ascontiguousarray(
        np.asarray(inputs_np, dtype=np.float32).reshape(B_FULL, N, C)
    )
    gam = np.asarray(gamma_np, dtype=np.float32).reshape(1, 1)
    ident = np.eye(C, dtype=np.float32).astype(ml_dtypes.bfloat16)
    in_maps = []
    for core in range(N_CORES):
        xs = x[core * B_LOC:(core + 1) * B_LOC].reshape(B_LOC * N, C)
        in_maps.append({
            "x": np.ascontiguousarray(xs),
            "gamma": gam,
            "ident": ident,
        })
    return in_maps


def kernel(inputs, gamma):
    nc = _get_nc()
    in_maps = make_in_maps(inputs, gamma)
    res = run_bass_kernel_spmd(nc, in_maps, core_ids=list(range(N_CORES)))
    outs = [np.asarray(res.results[c]["y"], dtype=np.float32)
            .reshape(B_LOC, N, C) for c in range(N_CORES)]
    y = np.concatenate(outs, axis=0).reshape(B_FULL, H, W, C)
    return y.astype(np.float32)
